# revision 8
# baseline (speedup 1.0000x reference)
"""RNN-T Joiner kernel for Trainium2, data-parallel over B across 8 NeuronCores.

Per core b: out[t,u,:] = relu(src[b,t,:] + tgt[b,u,:]) @ W.T + bias
  src (256,512) tgt (64,512) W (1024,512) bias (1024,) -> out (256,64,1024) f32

Layout: D (=512) lives on SBUF partitions (4 subtiles of 128) for both the
activation tiles and W, so the TensorE contracts over D. Activations are
produced directly in transposed [d, t] layout on the Scalar engine
(relu(srcT + tgtT[:,u]) with the tgt column as the per-partition bias), W is
transposed once at startup via PE-mode transpose. Matmuls run as float32r
(full PE rate at free dim 512). The Vector engine adds the output bias while
copying PSUM->SBUF, and DMA streams the 64 MiB per-core output back to HBM.
"""

import numpy as np

import concourse.bass as bass
import concourse.mybir as mybir
import concourse.tile as tile
from concourse.bass_utils import run_bass_kernel_spmd
from concourse.masks import make_identity
from concourse.vector_clock import ScopedClock

B, T, U, D, V = 8, 256, 64, 512, 1024
P = 128
KS = D // P   # 4 contraction subtiles
TS = T // P   # 2 t tiles
VC = V // 512  # 2 psum chunks
F32 = mybir.dt.float32
F32R = mybir.dt.float32r


def _patched_drain_and_barrier(self, tick_clock, wait_clock):
    # This walrus build rejects the multi-wait tail drain ("Too many sync
    # wait commands"); emit the waits as single-wait SP instructions instead.
    nc = self.nc
    probe = mybir.InstNoOp(name="tile-taildrain-probe", engine=mybir.EngineType.SP)
    wait_clock.add_sem_waits(probe, ScopedClock({None: tick_clock.global_clock}))
    si = probe.sync_info
    waits = list(si.on_wait) if si is not None and si.on_wait else []
    handles = {h.name: h for h in self.sems.allocated().values()}
    for w in waits:
        h = handles[w.ant_name]
        assert w.wait_mode == "sem-ge-imm", w.wait_mode
        nc.sync.wait_ge(h, w.wait_value)
    nc.sync.drain()
    nc.all_engine_barrier()
    popped = nc._tile_sem_poison_stack.pop()
    assert popped is self._sem_poison
    nc.clear_and_free_semaphores(list(self.sems.allocated().values()))
    nc.all_engine_barrier()


tile.TileContext._drain_and_barrier = _patched_drain_and_barrier


def _split_multi_waits(bir_json: bytes) -> bytes:
    # Same walrus limitation as above, for regular engine instructions: cap
    # embedded sync waits at 1, hoisting extras onto single-wait
    # EventSemaphore instructions inserted just before on the same engine.
    import json as _json

    bj = _json.loads(bir_json)
    for fn in bj["functions"]:
        for blk in fn["blocks"]:
            new = []
            for inst in blk["instructions"]:
                si = inst.get("sync_info")
                ow = (si or {}).get("on_wait") or []
                if len(ow) > 1:
                    for j, w in enumerate(ow[:-1]):
                        new.append({
                            "engine": inst["engine"],
                            "ins": [],
                            "outs": [],
                            "name": f"{inst['name']}-hw{j}",
                            "opcode": "EventSemaphore",
                            "sync_info": {"on_update": [], "on_wait": [w]},
                        })
                    si["on_wait"] = [ow[-1]]
                new.append(inst)
            blk["instructions"] = new
    return _json.dumps(bj).encode()


import concourse.bass2jax as _bass2jax

_orig_compile_bir_kernel = _bass2jax.compile_bir_kernel


def _compile_bir_kernel_split(bir_json, tmpdir, neff_name="file.neff"):
    return _orig_compile_bir_kernel(
        _split_multi_waits(bir_json), tmpdir, neff_name=neff_name
    )


_bass2jax.compile_bir_kernel = _compile_bir_kernel_split


def build_kernel() -> bass.Bass:
    nc = bass.Bass()
    src = nc.dram_tensor("src", [T, D], F32, kind="ExternalInput")
    tgt = nc.dram_tensor("tgt", [U, D], F32, kind="ExternalInput")
    W = nc.dram_tensor("W", [V, D], F32, kind="ExternalInput")
    bvec = nc.dram_tensor("b", [V], F32, kind="ExternalInput")
    out = nc.dram_tensor("out", [T, U, V], F32, kind="ExternalOutput")

    with tile.TileContext(nc) as tc:
        with (
            tc.tile_pool(name="const", bufs=1) as const,
            tc.tile_pool(name="stage", bufs=1) as stage,
            tc.tile_pool(name="tpsum", bufs=2, space="PSUM") as tpsum,
            tc.tile_pool(name="act", bufs=4) as act_pool,
            tc.tile_pool(name="osb", bufs=6) as out_pool,
            tc.tile_pool(name="psum", bufs=6, space="PSUM") as psum_pool,
        ):
            ident = const.tile([P, P], F32)
            make_identity(nc, ident)

            # --- transpose src (T,D) -> srcT [128, KS, T] ---
            src_nat = stage.tile([P, TS, D], F32, tag="srcnat")
            nc.sync.dma_start(out=src_nat[:], in_=src.rearrange("(ts p) d -> p ts d", p=P))
            srcT = const.tile([P, KS, T], F32)
            for ts in range(TS):
                for k in range(KS):
                    pt = tpsum.tile([P, P], F32, tag="tp")
                    nc.tensor.transpose(pt[:], src_nat[:, ts, k * P:(k + 1) * P], ident[:])
                    nc.vector.tensor_copy(out=srcT[:, k, ts * P:(ts + 1) * P], in_=pt[:])

            # --- transpose tgt (U,D) -> tgtT [128, KS, U] ---
            tgt_nat = stage.tile([U, D], F32, tag="tgtnat")
            nc.sync.dma_start(out=tgt_nat[:], in_=tgt[:])
            tgtT = const.tile([P, KS, U], F32)
            for k in range(KS):
                pt = tpsum.tile([P, U], F32, tag="tp")
                nc.tensor.transpose(pt[:], tgt_nat[:, k * P:(k + 1) * P], ident[:U, :U])
                nc.vector.tensor_copy(out=tgtT[:, k, :], in_=pt[:])

            # --- transpose W (V,D) -> WT [128, KS, V] ---
            w_nat = stage.tile([P, V // P, D], F32, tag="wnat")
            nc.sync.dma_start(out=w_nat[:], in_=W.rearrange("(vo p) d -> p vo d", p=P))
            WT = const.tile([P, KS, V], F32R)
            for vo in range(V // P):
                for k in range(KS):
                    pt = tpsum.tile([P, P], F32, tag="tp")
                    nc.tensor.transpose(pt[:], w_nat[:, vo, k * P:(k + 1) * P], ident[:])
                    nc.vector.tensor_copy(out=WT[:, k, vo * P:(vo + 1) * P], in_=pt[:])

            # --- broadcast bias across partitions: b_bcast [128, V] ---
            ones = const.tile([1, P], F32)
            nc.vector.memset(ones[:], 1.0)
            b_row = const.tile([1, V], F32)
            nc.sync.dma_start(out=b_row[:], in_=bvec[:].unsqueeze(0))
            b_bcast = const.tile([P, V], F32)
            for vc in range(VC):
                pb = tpsum.tile([P, 512], F32, tag="tp")
                nc.tensor.matmul(pb[:], lhsT=ones[:], rhs=b_row[:, vc * 512:(vc + 1) * 512],
                                 start=True, stop=True)
                nc.vector.tensor_copy(out=b_bcast[:, vc * 512:(vc + 1) * 512], in_=pb[:])

            # --- main loop over (t-tile, u) ---
            for ts in range(TS):
                for u in range(U):
                    actT = act_pool.tile([P, KS, P], F32R, tag="act")
                    for k in range(KS):
                        nc.scalar.activation(
                            out=actT[:, k, :],
                            in_=srcT[:, k, ts * P:(ts + 1) * P],
                            func=mybir.ActivationFunctionType.Relu,
                            bias=tgtT[:, k, u:u + 1],
                            scale=1.0,
                        )
                    psums = [
                        psum_pool.tile([P, 512], F32, tag="mm", name=f"mm{ts}_{u}_{vc}")
                        for vc in range(VC)
                    ]
                    for k in range(KS):
                        lhs = actT[:, k, :]
                        for vc in range(VC):
                            nc.tensor.matmul(
                                psums[vc][:],
                                lhsT=lhs,
                                rhs=WT[:, k, vc * 512:(vc + 1) * 512],
                                start=(k == 0),
                                stop=(k == KS - 1),
                            )
                    out_sb = out_pool.tile([P, V], F32, tag="osb")
                    for vc in range(VC):
                        nc.vector.tensor_add(
                            out=out_sb[:, vc * 512:(vc + 1) * 512],
                            in0=psums[vc][:],
                            in1=b_bcast[:, vc * 512:(vc + 1) * 512],
                        )
                    nc.sync.dma_start(
                        out=out[ts * P:(ts + 1) * P, u, :], in_=out_sb[:]
                    )
    return nc


_NC_CACHE = None


def kernel(source_encodings, source_lengths, target_encodings, target_lengths, W, b):
    global _NC_CACHE
    if _NC_CACHE is None:
        _NC_CACHE = build_kernel()
    nc = _NC_CACHE

    src = np.ascontiguousarray(np.asarray(source_encodings, dtype=np.float32))
    tgt = np.ascontiguousarray(np.asarray(target_encodings, dtype=np.float32))
    Wf = np.ascontiguousarray(np.asarray(W, dtype=np.float32))
    bf = np.ascontiguousarray(np.asarray(b, dtype=np.float32))

    in_maps = [
        {"src": src[i], "tgt": tgt[i], "W": Wf, "b": bf} for i in range(B)
    ]
    res = run_bass_kernel_spmd(nc, in_maps, list(range(B)))
    outs = np.stack([res.results[i]["out"] for i in range(B)], axis=0)
    return (
        outs,
        np.asarray(source_lengths, dtype=np.int32),
        np.asarray(target_lengths, dtype=np.int32),
    )
